# revision 1
# baseline (speedup 1.0000x reference)
"""Causal self-attention (B=4, T=1024, D=2048, H=16) on 8 trn2 NeuronCores.

Sharding: data-parallel over batch (4) x tensor-parallel over heads (2).
Core c handles batch b = c//2, head-half hh = c%2 (heads hh*8 .. hh*8+8).

Per-core plan (all matmuls float32r, fp32 PSUM accumulation):
  v      [t, c]  : lhsT = xT tile [k,t], rhs = wv [k,c]   (first, all heads)
  then per head h (pipelined):
    qT/kT [d, t] : lhsT = w_{q,k} tile [k,c=h], rhs = xT [k,t]
    sT    [tk,tq]: lhsT = kT block, rhs = qT slice (causal: tq >= 128*j only)
    pT    = exp(scale * sT) via ACT (no max-subtraction; |scaled scores| ~ 6)
    diag blocks masked multiplicatively with an upper-triangular 0/1 mask
    yT    [d, tq] += v_j-gemm: lhsT = v block, rhs = pT block (PSUM accum)
    r     [1, tq] += ones^T @ pT (softmax row sums, same rhs stream)
    yT_norm = yT * bcast(1/r) (DVE copy -> GpSimd bcast -> DVE approx-recip)
    pairwise AllGather of this head's yT (overlaps later heads' compute)
  out    [t, c_half] = yT_full-gemm against this half's w_proj columns
Host side: slice/transpose inputs per core, concat outputs (pure gather).
"""

import numpy as np

import concourse.bass as bass
import concourse.mybir as mybir
import concourse.tile as tile
from concourse import bacc
from concourse.bass_utils import run_bass_kernel_spmd

B, T, D = 4, 1024, 2048
H, DH = 16, 128
N_CORES = 8
TP = 2                      # head-halves per batch
HPC = H // TP               # heads per core = 8
CPC = HPC * DH              # channels per core = 1024
KC = D // 128               # contraction chunks = 16
SCALE = 1.0 / float(np.sqrt(DH))

F32 = mybir.dt.float32
F32R = mybir.dt.float32r

PAIRS = [[2 * i, 2 * i + 1] for i in range(B)]


def build_kernel():
    nc = bacc.Bacc("TRN2", target_bir_lowering=False, debug=False,
                   num_devices=N_CORES)

    xT_ap = nc.dram_tensor("xT", [D, T], F32R, kind="ExternalInput").ap()
    wq_ap = nc.dram_tensor("wq", [D, CPC], F32R, kind="ExternalInput").ap()
    wk_ap = nc.dram_tensor("wk", [D, CPC], F32R, kind="ExternalInput").ap()
    wv_ap = nc.dram_tensor("wv", [D, CPC], F32R, kind="ExternalInput").ap()
    wp_ap = nc.dram_tensor("wp", [D, CPC], F32R, kind="ExternalInput").ap()
    maskT_ap = nc.dram_tensor("maskT", [128, 128], F32R, kind="ExternalInput").ap()
    out_ap = nc.dram_tensor("out", [T, CPC], F32, kind="ExternalOutput").ap()

    with tile.TileContext(nc) as tc:
        _body(nc, tc, xT_ap, wq_ap, wk_ap, wv_ap, wp_ap, maskT_ap, out_ap)
    nc.compile()
    return nc


def _body(nc, tc, xT_ap, wq_ap, wk_ap, wv_ap, wp_ap, maskT_ap, out_ap):
    Exp = mybir.ActivationFunctionType.Exp
    mult = mybir.AluOpType.mult

    with tc.tile_pool(name="const", bufs=1) as const, \
         tc.tile_pool(name="dram", bufs=HPC, space="DRAM") as dram:
        maskT = const.tile([128, 128], F32R, tag="maskT")
        nc.sync.dma_start(out=maskT, in_=maskT_ap)
        ones_f32 = const.tile([128, 1], F32, tag="ones_f32")
        nc.vector.memset(ones_f32, 1.0)
        ones_col = const.tile([128, 1], F32R, tag="ones_col")
        nc.scalar.copy(out=ones_col, in_=ones_f32)

        yt_loc = [dram.tile([128, T], F32R, tag="ytl", name=f"ytl{h}")
                  for h in range(HPC)]
        yt_all = [dram.tile([TP, 128, T], F32R, tag="yta", name=f"yta{h}")
                  for h in range(HPC)]

        with tc.tile_pool(name="xa", bufs=KC) as xa, \
             tc.tile_pool(name="vvp", bufs=8) as vvp:
            xts = []
            for k in range(KC):
                xt = xa.tile([128, T], F32R, tag="xT", name=f"xt{k}")
                nc.sync.dma_start(out=xt, in_=xT_ap[128 * k:128 * (k + 1), :])
                xts.append(xt)
            vv = [vvp.tile([128, CPC], F32R, tag="vv", name=f"vv{j}")
                  for j in range(8)]

            # ---- v natural [t, c]: stationary xT slices, moving wv ----
            with tc.tile_pool(name="wv", bufs=3) as wvp, \
                 tc.tile_pool(name="pv", bufs=8, space="PSUM") as pv:
                for ch in range(2):
                    ps = [pv.tile([128, 512], F32, tag="pv",
                                  name=f"pv{ch}_{i}") for i in range(8)]
                    for k in range(KC):
                        wt = wvp.tile([128, 512], F32R, tag="wv",
                                      name=f"wv{ch}_{k}")
                        nc.scalar.dma_start(
                            out=wt,
                            in_=wv_ap[128 * k:128 * (k + 1),
                                      512 * ch:512 * (ch + 1)])
                        for tch in range(8):
                            nc.tensor.matmul(
                                ps[tch],
                                xts[k][:, 128 * tch:128 * (tch + 1)], wt,
                                start=(k == 0), stop=(k == KC - 1))
                    for tch in range(8):
                        nc.scalar.copy(
                            out=vv[tch][:, 512 * ch:512 * (ch + 1)],
                            in_=ps[tch])

            # ---- per-head: qk gemm + attention + per-head AllGather ----
            with tc.tile_pool(name="wqk", bufs=4) as wqk, \
                 tc.tile_pool(name="qkp", bufs=4) as qkp, \
                 tc.tile_pool(name="pt", bufs=10) as ptp, \
                 tc.tile_pool(name="yt", bufs=2) as yt_pool, \
                 tc.tile_pool(name="att_sm", bufs=2) as asm, \
                 tc.tile_pool(name="pa", bufs=2, space="PSUM") as pa, \
                 tc.tile_pool(name="ps_s", bufs=2, space="PSUM") as pss, \
                 tc.tile_pool(name="ps_y", bufs=2, space="PSUM") as psy, \
                 tc.tile_pool(name="ps_r", bufs=2, space="PSUM") as psr:
                def load_wqk(h2):
                    tiles = []
                    for (w_ap, nm) in ((wq_ap, "q"), (wk_ap, "k")):
                        wt = wqk.tile([128, KC, 128], F32R, tag="wqk",
                                      name=f"w{nm}{h2}")
                        nc.sync.dma_start(
                            out=wt,
                            in_=w_ap[:, 128 * h2:128 * (h2 + 1)].rearrange(
                                "(k p) j -> p k j", p=128))
                        tiles.append(wt)
                    return tiles

                wts_next = load_wqk(0)
                for h in range(HPC):
                    wts_cur = wts_next
                    if h + 1 < HPC:
                        wts_next = load_wqk(h + 1)
                    qkT = []
                    for wi, nm in ((0, "q"), (1, "k")):
                        outT = qkp.tile([128, T], F32R, tag="qkT",
                                        name=f"{nm}T{h}")
                        qkT.append(outT)
                        wt = wts_cur[wi]
                        for th in range(2):
                            ps = pa.tile([128, 512], F32, tag="pqk")
                            for k in range(KC):
                                nc.tensor.matmul(
                                    ps, wt[:, k, :],
                                    xts[k][:, 512 * th:512 * (th + 1)],
                                    start=(k == 0), stop=(k == KC - 1))
                            nc.scalar.copy(
                                out=outT[:, 512 * th:512 * (th + 1)], in_=ps)
                    qTh, kTh = qkT

                    pts = []
                    for j in range(8):
                        pt = ptp.tile([128, T], F32R, tag="pT",
                                      name=f"pT{h}_{j}")
                        pts.append(pt)
                        off = 128 * j
                        while off < T:
                            cw = min(512, T - off)
                            sp = pss.tile([128, 512], F32, tag="sT")
                            nc.tensor.matmul(
                                sp[:, :cw], kTh[:, 128 * j:128 * (j + 1)],
                                qTh[:, off:off + cw], start=True, stop=True)
                            nc.scalar.activation(
                                out=pt[:, off - 128 * j:off - 128 * j + cw],
                                in_=sp[:, :cw], func=Exp, scale=SCALE)
                            off += cw
                        # causal mask on the diagonal block (local cols 0:128)
                        nc.vector.tensor_tensor(
                            out=pt[:, 0:128], in0=pt[:, 0:128], in1=maskT,
                            op=mult)

                    yt = yt_pool.tile([128, T], F32R, tag="yT", name=f"yt{h}")
                    for g in range(2):
                        tq0 = 512 * g
                        jmax = 4 * (g + 1)
                        yp = psy.tile([128, 512], F32, tag="yp")
                        rp = psr.tile([1, 512], F32, tag="rp")
                        for j in range(jmax):
                            lo = max(tq0, 128 * j)          # first valid tq
                            w = tq0 + 512 - lo
                            rhs = pts[j][:, lo - 128 * j:lo - 128 * j + w]
                            vblk = vv[j][:, 128 * h:128 * (h + 1)]
                            nc.tensor.matmul(
                                yp[:, lo - tq0:lo - tq0 + w], vblk, rhs,
                                start=(j == 0), stop=(j == jmax - 1))
                            nc.tensor.matmul(
                                rp[:, lo - tq0:lo - tq0 + w], ones_col, rhs,
                                start=(j == 0), stop=(j == jmax - 1))
                        # softmax denom: psum -> sbuf -> bcast -> recip -> mult
                        r_sb = asm.tile([1, 512], F32, tag="r_sb")
                        nc.vector.tensor_copy(out=r_sb, in_=rp)
                        r_bc = asm.tile([128, 512], F32, tag="r_bc")
                        nc.gpsimd.partition_broadcast(r_bc, r_sb)
                        rec = asm.tile([128, 512], F32, tag="rec")
                        nc.vector.reciprocal_approx_fast(out=rec, in_=r_bc)
                        from concourse.dve_ops import RECIPROCAL_APPROX_NR
                        nc.vector._custom_dve(
                            RECIPROCAL_APPROX_NR, out=rec, in0=r_bc, in1=rec,
                            s0=2.0)
                        nc.vector.tensor_tensor(out=yt[:, tq0:tq0 + 512],
                                                in0=yp, in1=rec, op=mult)
                    # ship this head's yT to the pair as soon as it's done
                    nc.sync.dma_start(out=yt_loc[h], in_=yt)
                    nc.gpsimd.collective_compute(
                        "AllGather", mybir.AluOpType.bypass,
                        replica_groups=PAIRS,
                        ins=[yt_loc[h].opt()], outs=[yt_all[h].opt()])

            # ---- output projection out[t, c_half] = yT_full @ wp cols ----
            with tc.tile_pool(name="peer", bufs=2 * HPC) as peer_pool, \
                 tc.tile_pool(name="wp", bufs=4) as wpp, \
                 tc.tile_pool(name="out_sb", bufs=4) as osb, \
                 tc.tile_pool(name="ps_o", bufs=8, space="PSUM") as pso:
                yfull = []
                for r in range(TP):
                    for h2 in range(HPC):
                        t2 = peer_pool.tile([128, T], F32R, tag="yfull",
                                            name=f"yfull{r}_{h2}")
                        nc.sync.dma_start(out=t2, in_=yt_all[h2][r])
                        yfull.append(t2)
                for cc in range(2):      # 512-wide halves of my CPC out cols
                    ps = [pso.tile([128, 512], F32, tag="po",
                                   name=f"po{cc}_{m}") for m in range(8)]
                    for kk in range(KC):
                        wt = wpp.tile([128, 512], F32R, tag="wp",
                                      name=f"wp{cc}_{kk}")
                        nc.scalar.dma_start(
                            out=wt, in_=wp_ap[128 * kk:128 * (kk + 1),
                                              512 * cc:512 * (cc + 1)])
                        for m in range(8):
                            nc.tensor.matmul(
                                ps[m], yfull[kk][:, 128 * m:128 * (m + 1)],
                                wt, start=(kk == 0), stop=(kk == KC - 1))
                    for m in range(8):
                        ot = osb.tile([128, 512], F32, tag="ot")
                        nc.scalar.copy(out=ot, in_=ps[m])
                        nc.sync.dma_start(
                            out=out_ap[128 * m:128 * (m + 1),
                                       512 * cc:512 * (cc + 1)],
                            in_=ot)


_NC_CACHE = None


def _get_nc():
    global _NC_CACHE
    if _NC_CACHE is None:
        _NC_CACHE = build_kernel()
    return _NC_CACHE


def kernel(x, w_qkv, w_proj, _trace=False, _trace_kwargs=None):
    x = np.asarray(x, dtype=np.float32)
    w_qkv = np.asarray(w_qkv, dtype=np.float32)
    w_proj = np.asarray(w_proj, dtype=np.float32)

    maskT = np.triu(np.ones((128, 128), dtype=np.float32))

    in_maps = []
    for c in range(N_CORES):
        b, hh = c // TP, c % TP
        cols = slice(hh * CPC, (hh + 1) * CPC)
        in_maps.append({
            "xT": np.ascontiguousarray(x[b].T),
            "wq": np.ascontiguousarray(w_qkv[:, :D][:, cols]),
            "wk": np.ascontiguousarray(w_qkv[:, D:2 * D][:, cols]),
            "wv": np.ascontiguousarray(w_qkv[:, 2 * D:][:, cols]),
            "wp": np.ascontiguousarray(w_proj[:, cols]),
            "maskT": maskT,
        })

    nc = _get_nc()
    res = run_bass_kernel_spmd(nc, in_maps, list(range(N_CORES)),
                               trace=_trace, **(_trace_kwargs or {}))

    out = np.empty((B, T, D), dtype=np.float32)
    for c in range(N_CORES):
        b, hh = c // TP, c % TP
        out[b, :, hh * CPC:(hh + 1) * CPC] = res.results[c]["out"]
    if _trace:
        return out, res
    return out



# revision 10
# speedup vs baseline: 1.1171x; 1.1171x over previous
"""Causal self-attention (B=4, T=1024, D=2048, H=16) on 8 trn2 NeuronCores.

Sharding: data-parallel over batch (4) x tensor-parallel over heads (2).
Core c handles batch b = c//2, head-half hh = c%2 (heads hh*8 .. hh*8+8).

All matmuls in bf16 (fp32 PSUM accumulation); emulated end-to-end rel err
vs the fp32 reference is ~3.9e-3 (harness gate 2e-2).

Schedule (per core):
  phase F: 4 fused passes; pass p = (ch=p//2, tg=p%2) computes the v-gemm
    quarter [t-half tg, c-half ch] while head p's q/k gemms ride the same
    xts[k] stream (8 matmuls per k-step, 8 PSUM banks exactly).
  phase H: per head h: S (scores^T per tk-block j) -> exp on ACT (bf16 pt)
    -> R (row sums via ones-gemm) -> Y (v-gemm, PSUM accum) -> normalize on
    DVE -> per-head pair AllGather (Shared-HBM output). Heads 0-3 interleave
    head h+4's q/k gemms into the stream so PE never waits on exp.
  phase P: out-proj accumulating 16 chunks ordered so the last-arriving
    (head-7 gather) chunks come last; wp/yfull prefetched during phase H.

DMA queues: sync = x/wv/wqk (consumption order), scalar = wp/yfull
prefetch + yt/out writes.
"""

import numpy as np
import ml_dtypes

import concourse.bass as bass
import concourse.mybir as mybir
import concourse.tile as tile
from concourse import bacc
from concourse.bass_utils import run_bass_kernel_spmd

B, T, D = 4, 1024, 2048
H, DH = 16, 128
N_CORES = 8
TP = 2                      # head-halves per batch
HPC = H // TP               # heads per core = 8
CPC = HPC * DH              # channels per core = 1024
KC = D // 128               # contraction chunks = 16
SCALE = 1.0 / float(np.sqrt(DH))

F32 = mybir.dt.float32
CDT = mybir.dt.bfloat16     # compute dtype for all matmul operands
NP_CDT = ml_dtypes.bfloat16

PAIRS = [[2 * i, 2 * i + 1] for i in range(B)]


def build_kernel():
    nc = bacc.Bacc("TRN2", target_bir_lowering=False, debug=False,
                   num_devices=N_CORES)

    xT_ap = nc.dram_tensor("xT", [D, T], CDT, kind="ExternalInput").ap()
    wq_ap = nc.dram_tensor("wq", [D, CPC], CDT, kind="ExternalInput").ap()
    wk_ap = nc.dram_tensor("wk", [D, CPC], CDT, kind="ExternalInput").ap()
    wv_ap = nc.dram_tensor("wv", [D, CPC], CDT, kind="ExternalInput").ap()
    wp_ap = nc.dram_tensor("wp", [D, CPC], CDT, kind="ExternalInput").ap()
    maskT_ap = nc.dram_tensor("maskT", [128, 128], CDT, kind="ExternalInput").ap()
    out_ap = nc.dram_tensor("out", [T, CPC], F32, kind="ExternalOutput").ap()

    with tile.TileContext(nc) as tc:
        _body(nc, tc, xT_ap, wq_ap, wk_ap, wv_ap, wp_ap, maskT_ap, out_ap)
    nc.compile()
    return nc


def _body(nc, tc, xT_ap, wq_ap, wk_ap, wv_ap, wp_ap, maskT_ap, out_ap):
    Exp = mybir.ActivationFunctionType.Exp
    mult = mybir.AluOpType.mult
    from concourse.dve_ops import RECIPROCAL_APPROX_NR

    wp_t = {}
    yfull = {}
    pending_yfull = []

    with tc.tile_pool(name="const", bufs=1) as const, \
         tc.tile_pool(name="dram", bufs=HPC, space="DRAM") as dram, \
         tc.tile_pool(name="vvp", bufs=8) as vvp, \
         tc.tile_pool(name="wqk", bufs=8) as wqk, \
         tc.tile_pool(name="qkT", bufs=5) as qkp, \
         tc.tile_pool(name="pt", bufs=10) as ptp, \
         tc.tile_pool(name="yt", bufs=2) as ytp, \
         tc.tile_pool(name="att_sm", bufs=2) as asm:
        maskT = const.tile([128, 128], CDT, tag="maskT")
        nc.sync.dma_start(out=maskT, in_=maskT_ap)
        ones_col = const.tile([128, 1], CDT, tag="ones_col")
        nc.vector.memset(ones_col, 1.0)

        yt_loc = [dram.tile([128, T], CDT, tag="ytl", name=f"ytl{h}")
                  for h in range(HPC)]
        yt_all = [dram.tile([TP, 128, T], CDT, tag="yta", name=f"yta{h}")
                  for h in range(HPC)]

        wq_t, wk_t = {}, {}

        def load_wqk(h2):
            for (w_ap, store) in ((wq_ap, wq_t), (wk_ap, wk_t)):
                wt = wqk.tile([128, KC, 128], CDT, tag="wqk",
                              name=f"wqk{h2}_{0 if store is wq_t else 1}")
                nc.sync.dma_start(
                    out=wt,
                    in_=w_ap[:, 128 * h2:128 * (h2 + 1)].rearrange(
                        "(k p) j -> p k j", p=128))
                store[h2] = wt

        vv = [vvp.tile([128, CPC], CDT, tag="vv", name=f"vv{j}")
              for j in range(8)]
        qT, kT = {}, {}

        def head_block(h, gen, pss, psy, psr):
            """Emit head h's S/exp/R/Y/normalize/gather, pumping `gen`
            (one PE/copy instruction per next()) into the stalls."""
            def pump(n):
                for _ in range(n):
                    f = next(gen, None)
                    if f is not None:
                        f()

            qTh, kTh = qT[h], kT[h]
            pts = []
            for j in range(8):
                pt = ptp.tile([128, T], CDT, tag="pT", name=f"pT{h}_{j}")
                pts.append(pt)
                off = 128 * j
                first = True
                while off < T:
                    w = min(512, T - off)
                    sp = pss.tile([128, 512], F32, tag="sT", name=f"sT{h}")
                    nc.tensor.matmul(
                        sp[:, :w], kTh[:, 128 * j:128 * (j + 1)],
                        qTh[:, off:off + w], start=True, stop=True)
                    nc.scalar.activation(
                        out=pt[:, off - 128 * j:off - 128 * j + w],
                        in_=sp[:, :w], func=Exp, scale=SCALE)
                    if first:
                        nc.vector.tensor_tensor(
                            out=pt[:, 0:128], in0=pt[:, 0:128],
                            in1=maskT, op=mult)
                        first = False
                    pump(2)
                    off += w

            yt = ytp.tile([128, T], CDT, tag="yT", name=f"yt{h}")
            for g in range(2):
                tq0 = 512 * g
                jmax = 4 * (g + 1)
                # row sums first so the reciprocal chain overlaps Y
                rp = psr.tile([1, 512], F32, tag="rp", name=f"rp{h}")
                for j in range(jmax):
                    lo = max(tq0, 128 * j)
                    w = tq0 + 512 - lo
                    rhs = pts[j][:, lo - 128 * j:lo - 128 * j + w]
                    nc.tensor.matmul(
                        rp[:, lo - tq0:lo - tq0 + w], ones_col, rhs,
                        start=(j == 0), stop=(j == jmax - 1))
                    pump(2)
                r_sb = asm.tile([1, 512], F32, tag="r_sb")
                nc.vector.tensor_copy(out=r_sb, in_=rp)
                r_bc = asm.tile([128, 512], F32, tag="r_bc")
                nc.gpsimd.partition_broadcast(r_bc, r_sb)
                rec = asm.tile([128, 512], F32, tag="rec")
                nc.vector.reciprocal_approx_fast(out=rec, in_=r_bc)
                nc.vector._custom_dve(
                    RECIPROCAL_APPROX_NR, out=rec, in0=r_bc, in1=rec,
                    s0=2.0)
                yp = psy.tile([128, 512], F32, tag="yp", name=f"yp{h}")
                for j in range(jmax):
                    lo = max(tq0, 128 * j)
                    w = tq0 + 512 - lo
                    rhs = pts[j][:, lo - 128 * j:lo - 128 * j + w]
                    nc.tensor.matmul(
                        yp[:, lo - tq0:lo - tq0 + w],
                        vv[j][:, 128 * h:128 * (h + 1)], rhs,
                        start=(j == 0), stop=(j == jmax - 1))
                    pump(2)
                nc.vector.tensor_tensor(out=yt[:, tq0:tq0 + 512],
                                        in0=yp, in1=rec, op=mult)
            pump(8)
            nc.scalar.dma_start(out=yt_loc[h], in_=yt)
            nc.gpsimd.collective_compute(
                "AllGather", mybir.AluOpType.bypass,
                replica_groups=PAIRS,
                ins=[yt_loc[h].opt()], outs=[yt_all[h].opt()])
            pending_yfull.append(h)

        with tc.tile_pool(name="xa", bufs=KC) as xa:
            # ---- DMA: phase-F consumption order on the sync queue ----
            load_wqk(0)
            load_wqk(1)
            xts = []
            wv_t = {}
            with tc.tile_pool(name="wvt", bufs=2 * KC) as wvt:
                for k in range(KC):
                    xt = xa.tile([128, T], CDT, tag="xT", name=f"xt{k}")
                    nc.sync.dma_start(out=xt,
                                      in_=xT_ap[128 * k:128 * (k + 1), :])
                    xts.append(xt)
                    wt = wvt.tile([128, 512], CDT, tag="wv", name=f"wv0_{k}")
                    nc.sync.dma_start(
                        out=wt, in_=wv_ap[128 * k:128 * (k + 1), 0:512])
                    wv_t[(0, k)] = wt
                load_wqk(2)
                for k in range(KC):
                    wt = wvt.tile([128, 512], CDT, tag="wv", name=f"wv1_{k}")
                    nc.sync.dma_start(
                        out=wt, in_=wv_ap[128 * k:128 * (k + 1), 512:1024])
                    wv_t[(1, k)] = wt
                load_wqk(3)
                load_wqk(4)

                # ---- phase F: v quarters fused with QK of heads 0-3 ----
                with tc.tile_pool(name="ps_v", bufs=4, space="PSUM") as pv, \
                     tc.tile_pool(name="ps_qkF", bufs=4, space="PSUM") as pqf:
                    for p in range(4):
                        ch, tg, h = p // 2, p % 2, p
                        vps = [pv.tile([128, 512], F32, tag="pv",
                                       name=f"pv{p}_{i}") for i in range(4)]
                        qps = [pqf.tile([128, 512], F32, tag="pqk",
                                        name=f"pq{p}_{i}") for i in range(2)]
                        kps = [pqf.tile([128, 512], F32, tag="pqk",
                                        name=f"pk{p}_{i}") for i in range(2)]
                        for k in range(KC):
                            st, sp = (k == 0), (k == KC - 1)
                            for i in range(4):
                                tch = 4 * tg + i
                                nc.tensor.matmul(
                                    vps[i],
                                    xts[k][:, 128 * tch:128 * (tch + 1)],
                                    wv_t[(ch, k)], start=st, stop=sp)
                                if i < 2:
                                    nc.tensor.matmul(
                                        qps[i], wq_t[h][:, k, :],
                                        xts[k][:, 512 * i:512 * (i + 1)],
                                        start=st, stop=sp)
                                else:
                                    th = i - 2
                                    nc.tensor.matmul(
                                        kps[th], wk_t[h][:, k, :],
                                        xts[k][:, 512 * th:512 * (th + 1)],
                                        start=st, stop=sp)
                        for i in range(4):
                            tch = 4 * tg + i
                            nc.scalar.copy(
                                out=vv[tch][:, 512 * ch:512 * (ch + 1)],
                                in_=vps[i])
                        qTh = qkp.tile([128, T], CDT, tag="qT", name=f"qT{h}")
                        kTh = qkp.tile([128, T], CDT, tag="kT", name=f"kT{h}")
                        for th in range(2):
                            nc.scalar.copy(out=qTh[:, 512 * th:512 * (th + 1)],
                                           in_=qps[th])
                            nc.scalar.copy(out=kTh[:, 512 * th:512 * (th + 1)],
                                           in_=kps[th])
                        qT[h], kT[h] = qTh, kTh

            # ---- phase H part 1: heads 0-3, QK(h+4) interleaved ----
            with tc.tile_pool(name="ps_s", bufs=2, space="PSUM") as pss, \
                 tc.tile_pool(name="ps_qkH", bufs=2, space="PSUM") as pqh, \
                 tc.tile_pool(name="ps_y", bufs=2, space="PSUM") as psy, \
                 tc.tile_pool(name="ps_r", bufs=2, space="PSUM") as psr:

                def qk_gen(h2):
                    qTh = qkp.tile([128, T], CDT, tag="qT", name=f"qT{h2}")
                    kTh = qkp.tile([128, T], CDT, tag="kT", name=f"kT{h2}")
                    qT[h2], kT[h2] = qTh, kTh
                    for w_t, outT in ((wq_t[h2], qTh), (wk_t[h2], kTh)):
                        for th in range(2):
                            ps = pqh.tile([128, 512], F32, tag="pqk2",
                                          name=f"pqh{h2}_{th}")
                            for k in range(KC):
                                yield (lambda ps=ps, w=w_t, k=k, th=th:
                                       nc.tensor.matmul(
                                           ps, w[:, k, :],
                                           xts[k][:, 512 * th:512 * (th + 1)],
                                           start=(k == 0), stop=(k == KC - 1)))
                            yield (lambda ps=ps, o=outT, th=th:
                                   nc.scalar.copy(
                                       out=o[:, 512 * th:512 * (th + 1)],
                                       in_=ps))

                for h in range(4):
                    if h + 5 <= 7:
                        load_wqk(h + 5)
                    head_block(h, qk_gen(h + 4), pss, psy, psr)

        # xa closed: xts freed; open proj prefetch pools
        with tc.tile_pool(name="wpt", bufs=2 * KC) as wpt, \
             tc.tile_pool(name="yfull", bufs=2 * HPC) as yfp, \
             tc.tile_pool(name="osb", bufs=2) as osb:

            def issue_prefetch(h):
                # 8 wp chunks per head for heads 4-7 -> all 32 by head 7
                for i in range(8):
                    idx = 8 * (h - 4) + i
                    cc, kk = idx // KC, idx % KC
                    wt = wpt.tile([128, 512], CDT, tag="wp",
                                  name=f"wp{cc}_{kk}")
                    nc.scalar.dma_start(
                        out=wt, in_=wp_ap[128 * kk:128 * (kk + 1),
                                          512 * cc:512 * (cc + 1)])
                    wp_t[(cc, kk)] = wt
                while pending_yfull:
                    hd = pending_yfull.pop(0)
                    for r in range(TP):
                        t2 = yfp.tile([128, T], CDT, tag="yfull",
                                      name=f"yfull{r}_{hd}")
                        nc.scalar.dma_start(out=t2, in_=yt_all[hd][r])
                        yfull[(r, hd)] = t2

            # ---- phase H part 2: heads 4-7 ----
            with tc.tile_pool(name="ps_s2", bufs=2, space="PSUM") as pss, \
                 tc.tile_pool(name="ps_y2", bufs=2, space="PSUM") as psy, \
                 tc.tile_pool(name="ps_r2", bufs=2, space="PSUM") as psr:
                for h in range(4, HPC):
                    issue_prefetch(h)
                    head_block(h, iter(()), pss, psy, psr)

            # ---- phase P: output projection, head-7 chunks last ----
            while pending_yfull:
                hd = pending_yfull.pop(0)
                for r in range(TP):
                    t2 = yfp.tile([128, T], CDT, tag="yfull",
                                  name=f"yfull{r}_{hd}")
                    nc.scalar.dma_start(out=t2, in_=yt_all[hd][r])
                    yfull[(r, hd)] = t2
            ord_chunks = [(r, h) for h in range(HPC - 1) for r in range(TP)]
            ord_chunks += [(0, HPC - 1), (1, HPC - 1)]
            with tc.tile_pool(name="ps_o", bufs=8, space="PSUM") as pso:
                for cc in range(2):
                    ps = [pso.tile([128, 512], F32, tag="po",
                                   name=f"po{cc}_{m}") for m in range(8)]
                    for gi, (r, h) in enumerate(ord_chunks):
                        kk = r * HPC + h
                        wt = wp_t[(cc, kk)]
                        yf = yfull[(r, h)]
                        st, sp = (gi == 0), (gi == len(ord_chunks) - 1)
                        for m in range(8):
                            nc.tensor.matmul(
                                ps[m], yf[:, 128 * m:128 * (m + 1)], wt,
                                start=st, stop=sp)
                    for m in range(8):
                        ot = osb.tile([128, 512], F32, tag="ot")
                        nc.vector.tensor_copy(out=ot, in_=ps[m])
                        nc.scalar.dma_start(
                            out=out_ap[128 * m:128 * (m + 1),
                                       512 * cc:512 * (cc + 1)],
                            in_=ot)


_NC_CACHE = None


def _get_nc():
    global _NC_CACHE
    if _NC_CACHE is None:
        _NC_CACHE = build_kernel()
    return _NC_CACHE


def kernel(x, w_qkv, w_proj, _trace=False, _trace_kwargs=None):
    x = np.asarray(x, dtype=np.float32)
    w_qkv = np.asarray(w_qkv, dtype=np.float32)
    w_proj = np.asarray(w_proj, dtype=np.float32)

    maskT = np.triu(np.ones((128, 128), dtype=np.float32)).astype(NP_CDT)

    in_maps = []
    for c in range(N_CORES):
        b, hh = c // TP, c % TP
        cols = slice(hh * CPC, (hh + 1) * CPC)
        in_maps.append({
            "xT": np.ascontiguousarray(x[b].T).astype(NP_CDT),
            "wq": np.ascontiguousarray(w_qkv[:, :D][:, cols]).astype(NP_CDT),
            "wk": np.ascontiguousarray(w_qkv[:, D:2 * D][:, cols]).astype(NP_CDT),
            "wv": np.ascontiguousarray(w_qkv[:, 2 * D:][:, cols]).astype(NP_CDT),
            "wp": np.ascontiguousarray(w_proj[:, cols]).astype(NP_CDT),
            "maskT": maskT,
        })

    nc = _get_nc()
    res = run_bass_kernel_spmd(nc, in_maps, list(range(N_CORES)),
                               trace=_trace, **(_trace_kwargs or {}))

    out = np.empty((B, T, D), dtype=np.float32)
    for c in range(N_CORES):
        b, hh = c // TP, c % TP
        out[b, :, hh * CPC:(hh + 1) * CPC] = res.results[c]["out"]
    if _trace:
        return out, res
    return out
